# revision 14
# baseline (speedup 1.0000x reference)
"""Grouped-query attention (B=2,T=2048,D=2048, 4 groups x 4 heads x 128d) on 8 trn2 cores.

Sharding: core = (batch b, group g); b = core//4, g = core%4. Each core computes its
group's QKV projections, QK-rmsnorm+rope, causal flash-style attention, and a partial
output projection o_g @ wo_g; the host sums the 4 per-group partials per batch.

v2 changes over the 306us v1 (all DMA-issue + PE-overhead driven):
  - DMA issue is the phase-1 limiter (~608ns serial issue per dma_start on the Sync
    queue): weights are host-packed into 4KB-row blobs (wkv [128,4096], wq head-major
    [128,8192], misc cos|sin|mask|ident, gqk f32) so all inputs issue in ~27 DMAs
    ordered wkv, xt0..15, wq_h0, misc, gqk, wq_h1..3 -- the xt stream is never
    issue-starved and Q weights land right as K/V proj drains.
  - causal mask moved OFF the PE: post-exp DVE multiply of the diagonal 128-block
    with a 0/1 mask ([128,256] both heads via 3D AP). Kills 64 mask matmuls + 64
    maskt LDWEIGHTS + kb stationary thrash.
  - Q projection kc-outer per head (stationary held 4 matmuls; 64 LDW not 256).
  - output stores staged per-tb [128,2048] -> 16 out DMAs instead of 64 (tail was
    serialized on out-DMA issue).
  - rope swaps stay SBUF->SBUF DMAs on the Sync queue, which is idle after the
    input burst.

Device layout (per core): xt [D,T] bf16 = x[b].T; projections produce qT/kT/vT [n,T]
with head-dim on partitions -> QK^T directly (scores [j,i]); softmax denominator =
ones-matmul chains on PE; PV uses v natural (PE-transposed) as lhsT. All matmuls bf16
with f32 PSUM. Softmax without max-subtraction: |scores| <= sqrt(128) after rmsnorm.
"""

import sys
from contextlib import ExitStack

for _p in ("/opt/trn_rl_repo", "/opt/pypackages"):
    if _p not in sys.path:
        sys.path.insert(0, _p)

import numpy as np
import ml_dtypes

import concourse.bass as bass
import concourse.mybir as mybir
import concourse.tile as tile
from concourse import bacc
from concourse.bass_utils import run_bass_kernel_spmd

bf16 = ml_dtypes.bfloat16
BF = mybir.dt.bfloat16
F32 = mybir.dt.float32
AF = mybir.ActivationFunctionType

B, T, D = 2, 2048, 2048
HD, H, G = 128, 4, 4
KC = D // 128          # 16 contraction chunks
TB = T // 128          # 16 t blocks
IC = T // 512          # 4 i chunks
EPS = 1e-6
MULT2 = float(HD) ** -0.5   # mult^2 folded into q gains

# misc blob column layout: mask_pair | ident | maskt | cos | sin
# (small part [0:512] DMAs first)
MK0, ID0, MT0, COS0, SIN0 = 0, 256, 384, 512, 2560
MISC_W = 4608

_NC_CACHE = {}


def _halves(t, off, width, half=512):
    """3D view of a [128, 2*half] tile: [128, 2, width] starting at `off` in
    each half (covers both heads of a paired tile in one instruction)."""
    return bass.AP(tensor=t.tensor, offset=t.offset + off,
                   ap=[list(t.ap[0]), [half, 2], [1, width]])


def _build_nc():
    nc = bacc.Bacc(None)

    xt_d = nc.declare_dram_parameter("xt", [D, T], BF, isOutput=False)
    wkv_d = nc.declare_dram_parameter("wkv", [128, 2 * D], BF, isOutput=False)
    wq_d = nc.declare_dram_parameter("wqp", [128, H * D], BF, isOutput=False)
    misc_d = nc.declare_dram_parameter("misc", [128, MISC_W], BF, isOutput=False)
    gqk_d = nc.declare_dram_parameter("gqk", [128, 5], F32, isOutput=False)
    wo_d = nc.declare_dram_parameter("wop", [128, H * D], BF, isOutput=False)
    out_d = nc.declare_dram_parameter("out", [T, D], BF, isOutput=True)

    with tile.TileContext(nc) as tc:
        with ExitStack() as outer:
            persist = outer.enter_context(tc.tile_pool(name="persist", bufs=1))
            qhat = [persist.tile([128, T], BF, tag=f"qhat{h}", name=f"qhat{h}") for h in range(H)]
            khat = persist.tile([128, T], BF, tag="khat", name="khat")
            vnat = persist.tile([128, T], BF, tag="vnat", name="vnat")  # [j-local, tb*128+d]
            misc = persist.tile([128, MISC_W], BF, tag="misc", name="misc")
            gqk = persist.tile([128, 5], F32, tag="gqk", name="gqk")
            ones_bf = persist.tile([128, 1], BF, tag="ones", name="ones")
            eps_t = persist.tile([1, 1], F32, tag="eps", name="eps")

            nc.vector.memset(ones_bf, 1.0)
            nc.vector.memset(eps_t, EPS)

            # ---------------- Phase 1: projections + rmsnorm + rope ----------------
            with ExitStack() as s1:
                xt_p = s1.enter_context(tc.tile_pool(name="xt", bufs=1))
                w_p = s1.enter_context(tc.tile_pool(name="w", bufs=1))
                tmp_p = s1.enter_context(tc.tile_pool(name="tmp", bufs=1))
                row_p = s1.enter_context(tc.tile_pool(name="rows", bufs=1))

                xtall = xt_p.tile([128, KC * T], BF, tag="xtall", name="xtall")
                wkv = w_p.tile([128, 2 * D], BF, tag="wkv", name="wkv")
                wqp = w_p.tile([128, H * D], BF, tag="wqp", name="wqp")

                # DMA issue order (one serial issue queue): first K/V weights for
                # kc 0-3 + mask/ident (tiny), then the xt stream (kc=0 split for
                # first-matmul latency), rest of wkv, then Q head 0 weights,
                # cos/sin + gains, then the rest of wq.
                nc.sync.dma_start(out=wkv[:, 0:512], in_=wkv_d[:, 0:512])
                nc.sync.dma_start(out=wkv[:, D:D + 512], in_=wkv_d[:, D:D + 512])
                for q in range(4):
                    nc.sync.dma_start(out=xtall[:, q * 512:(q + 1) * 512],
                                      in_=xt_d[0:128, q * 512:(q + 1) * 512])
                for kc in range(1, 3):
                    nc.sync.dma_start(out=xtall[:, kc * T:(kc + 1) * T],
                                      in_=xt_d[kc * 128:(kc + 1) * 128, :])
                nc.sync.dma_start(out=wkv[:, 512:D], in_=wkv_d[:, 512:D])
                nc.sync.dma_start(out=wkv[:, D + 512:2 * D], in_=wkv_d[:, D + 512:2 * D])
                for kc in range(3, KC):
                    nc.sync.dma_start(out=xtall[:, kc * T:(kc + 1) * T],
                                      in_=xt_d[kc * 128:(kc + 1) * 128, :])
                nc.sync.dma_start(out=wqp[:, 0:D], in_=wq_d[:, 0:D])
                nc.sync.dma_start(out=misc[:, 0:COS0], in_=misc_d[:, 0:COS0])
                nc.sync.dma_start(out=misc[:, COS0:MISC_W], in_=misc_d[:, COS0:MISC_W])
                nc.sync.dma_start(out=gqk, in_=gqk_d[:, :])
                for h in range(1, H):
                    nc.sync.dma_start(out=wqp[:, h * D:(h + 1) * D],
                                      in_=wq_d[:, h * D:(h + 1) * D])

                def norm_tiles(nm):
                    sq = tmp_p.tile([128, T], BF, tag="sq", name=f"sq_{nm}", bufs=2)
                    gt = tmp_p.tile([128, T], BF, tag="gt", name=f"gt_{nm}", bufs=2)
                    sw = tmp_p.tile([128, T], BF, tag="sw", name=f"sw_{nm}", bufs=2)
                    t1 = tmp_p.tile([128, T], BF, tag="t1", name=f"t1_{nm}", bufs=2)
                    rb = tmp_p.tile([128, T], F32, tag="rb", name=f"rb_{nm}", bufs=2)
                    return sq, gt, sw, t1, rb

                def consume_chunk(ps, sq, gt, sw, t1, gain_col, tf):
                    """square + gain-mul + rotate-half rope, all 512-col chunked."""
                    sl = slice(tf * 512, (tf + 1) * 512)
                    nc.scalar.square(out=sq[:, sl], in_=ps)
                    nc.vector.tensor_scalar_mul(gt[:, sl], ps, gain_col)
                    nc.sync.dma_start(out=sw[0:64, sl], in_=gt[64:128, sl])
                    nc.sync.dma_start(out=sw[64:128, sl], in_=gt[0:64, sl])
                    nc.vector.tensor_mul(t1[:, sl], gt[:, sl],
                                         misc[:, COS0 + tf * 512:COS0 + (tf + 1) * 512])
                    nc.vector.tensor_mul(sw[:, sl], sw[:, sl],
                                         misc[:, SIN0 + tf * 512:SIN0 + (tf + 1) * 512])
                    nc.vector.tensor_add(t1[:, sl], t1[:, sl], sw[:, sl])

                def norm_pe_and_fin(nm, sq, t1, rb, hat_out):
                    """per-chunk: ones-mm -> sqrt -> recip -> gpsimd bcast -> hat.
                    hat lags one chunk so it never waits on the broadcast."""
                    srow = row_p.tile([1, T], F32, tag="srow", name=f"srow_{nm}", bufs=2)
                    prev = None
                    for tf in range(4):
                        sl = slice(tf * 512, (tf + 1) * 512)
                        pr = ps_row.tile([1, 512], F32, tag="ps_row", name=f"pr_{nm}{tf}")
                        nc.tensor.matmul(pr, ones_bf, sq[:, sl], start=True, stop=True)
                        nc.scalar.activation(out=srow[:, sl], in_=pr, func=AF.Sqrt,
                                             bias=eps_t[:, 0:1], scale=1.0 / HD)
                        nc.vector.reciprocal_approx_fast(out=srow[:, sl],
                                                         in_=srow[:, sl])
                        nc.gpsimd.partition_broadcast(rb[:, sl], srow[:, sl],
                                                      channels=128)
                        if prev is not None:
                            nc.vector.tensor_mul(hat_out[:, prev], t1[:, prev],
                                                 rb[:, prev])
                        prev = sl
                    nc.vector.tensor_mul(hat_out[:, prev], t1[:, prev], rb[:, prev])

                # ---- K+V projections, kc-outer interleaved (xt-stream paced) ----
                with ExitStack() as s1a:
                    ps_kv = s1a.enter_context(tc.tile_pool(name="ps_kv", bufs=1, space="PSUM"))
                    psK = [ps_kv.tile([128, 512], F32, tag=f"psK{tf}", name=f"psK{tf}")
                           for tf in range(4)]
                    psV = [ps_kv.tile([128, 512], F32, tag=f"psV{tf}", name=f"psV{tf}")
                           for tf in range(4)]
                    for kc in range(KC):
                        x0 = kc * T
                        for tf in range(4):
                            nc.tensor.matmul(psK[tf], wkv[:, kc * 128:(kc + 1) * 128],
                                             xtall[:, x0 + tf * 512:x0 + (tf + 1) * 512],
                                             start=(kc == 0), stop=(kc == KC - 1))
                        for tf in range(4):
                            nc.tensor.matmul(psV[tf], wkv[:, D + kc * 128:D + (kc + 1) * 128],
                                             xtall[:, x0 + tf * 512:x0 + (tf + 1) * 512],
                                             start=(kc == 0), stop=(kc == KC - 1))
                    sqK, gtK, swK, t1K, rbK = norm_tiles("K")
                    vtr = tmp_p.tile([128, T], BF, tag="vtr", name="vtr")
                    for tf in range(4):
                        consume_chunk(psK[tf], sqK, gtK, swK, t1K, gqk[:, 4:5], tf)
                    for tf in range(4):
                        nc.scalar.activation(out=vtr[:, tf * 512:(tf + 1) * 512],
                                             in_=psV[tf], func=AF.Copy)

                # ---- Q projections kc-outer per head + staggered norms + V transpose ----
                with ExitStack() as s1b:
                    ps_q = s1b.enter_context(tc.tile_pool(name="ps_q", bufs=4, space="PSUM"))
                    ps_row = s1b.enter_context(tc.tile_pool(name="ps_row", bufs=2, space="PSUM"))
                    ps_tp = s1b.enter_context(tc.tile_pool(name="ps_tp", bufs=2, space="PSUM"))

                    qn = [norm_tiles(f"Q{h}") for h in range(H)]

                    def proj_q(h):
                        sq, gt, sw, t1, rb = qn[h]
                        psqs = [ps_q.tile([128, 512], F32, tag="ps_q",
                                          name=f"psq{h}{tf}") for tf in range(4)]
                        for kc in range(KC):
                            w0 = h * D + kc * 128
                            x0 = kc * T
                            for tf in range(4):
                                nc.tensor.matmul(psqs[tf], wqp[:, w0:w0 + 128],
                                                 xtall[:, x0 + tf * 512:x0 + (tf + 1) * 512],
                                                 start=(kc == 0), stop=(kc == KC - 1))
                        for tf in range(4):
                            consume_chunk(psqs[tf], sq, gt, sw, t1, gqk[:, h:h + 1], tf)

                    def vtrans(r):
                        for tb in range(4 * r, 4 * r + 4):
                            pt_ = ps_tp.tile([128, 128], BF, tag="ps_tp", name=f"ps_tp{tb}")
                            nc.tensor.transpose(pt_, vtr[:, tb * 128:(tb + 1) * 128],
                                                misc[:, ID0:ID0 + 128])
                            nc.scalar.activation(out=vnat[:, tb * 128:(tb + 1) * 128],
                                                  in_=pt_, func=AF.Copy)

                    # Q0 projection leads: the K norm chain can only start once
                    # the last xt chunk lands, and putting it first would block
                    # the in-order PE queue on the ACT squares.
                    proj_q(0)
                    norm_pe_and_fin("K", sqK, t1K, rbK, khat)
                    vtrans(0)
                    proj_q(1)
                    norm_pe_and_fin("Q0", qn[0][0], qn[0][3], qn[0][4], qhat[0])
                    vtrans(1)
                    proj_q(2)
                    norm_pe_and_fin("Q1", qn[1][0], qn[1][3], qn[1][4], qhat[1])
                    vtrans(2)
                    proj_q(3)
                    norm_pe_and_fin("Q2", qn[2][0], qn[2][3], qn[2][4], qhat[2])
                    vtrans(3)
                    norm_pe_and_fin("Q3", qn[3][0], qn[3][3], qn[3][4], qhat[3])
                    # prewarm the EXP table so the first attention exp doesn't
                    # pay the 1.3us table switch on the QK->exp critical path
                    warm = row_p.tile([1, 1], F32, tag="warm", name="warm")
                    nc.scalar.activation(out=warm, in_=eps_t, func=AF.Exp)

            # ------- Phases 2+3: causal attention with pipelined output projection ---
            with ExitStack() as s2:
                o_p = s2.enter_context(tc.tile_pool(name="op", bufs=1))
                oT = [o_p.tile([128, T], BF, tag=f"oT{h}", name=f"oT{h}") for h in range(H)]
                wo_p = s2.enter_context(tc.tile_pool(name="wo", bufs=1))
                p_p = s2.enter_context(tc.tile_pool(name="pexp", bufs=26))
                pp_p = s2.enter_context(tc.tile_pool(name="ppair", bufs=8))
                dn_p = s2.enter_context(tc.tile_pool(name="dn", bufs=2))
                db_p = s2.enter_context(tc.tile_pool(name="dnb", bufs=4))
                ost_p = s2.enter_context(tc.tile_pool(name="ost", bufs=3))
                # rotating pool of 3x 2-bank tiles: paired score tiles,
                # denominator rows, and oproj psums all cycle through it.
                # Created before ps_po so its first slots map onto the psum
                # banks freed earliest at the phase-1 handoff.
                ps_sc = s2.enter_context(tc.tile_pool(name="ps_sc", bufs=3, space="PSUM"))
                ps_po = s2.enter_context(tc.tile_pool(name="ps_po", bufs=2, space="PSUM"))

                wo = wo_p.tile([128, H * D], BF, tag="wo", name="wo")
                nc.sync.dma_start(out=wo, in_=wo_d[:, :])

                def qk_exp(ic, pi, jb):
                    """2 QK matmuls into a paired [128,1024] tile, one 3D-AP exp,
                    DVE 0/1-mask multiply on the diagonal block."""
                    h0, h1 = 2 * pi, 2 * pi + 1
                    off = max(0, 128 * (jb - 4 * ic))
                    i0 = ic * 512
                    kb = khat[:, jb * 128:(jb + 1) * 128]
                    diag = jb >= 4 * ic
                    # hybrid causal mask: for the first two i-chunks the DVE
                    # queue is still digesting the Q-norm chains, so the mask is
                    # applied on the PE (accumulated -1e9 matmul); later chunks
                    # use a post-exp DVE 0/1-mask multiply.
                    pe_mask = diag and ic >= 2
                    sc = ps_sc.tile([128, 1024], F32, tag="sc", name=f"sc{ic}{pi}{jb}")
                    nc.tensor.matmul(sc[:, off:512], kb, qhat[h0][:, i0 + off:i0 + 512],
                                     start=True, stop=not pe_mask)
                    nc.tensor.matmul(sc[:, 512 + off:1024], kb,
                                     qhat[h1][:, i0 + off:i0 + 512],
                                     start=True, stop=not pe_mask)
                    if pe_mask:
                        nc.tensor.matmul(sc[:, off:off + 128],
                                         misc[:, MT0:MT0 + 128],
                                         misc[:, ID0:ID0 + 128],
                                         start=False, stop=True, skip_group_check=True)
                        nc.tensor.matmul(sc[:, 512 + off:512 + off + 128],
                                         misc[:, MT0:MT0 + 128],
                                         misc[:, ID0:ID0 + 128],
                                         start=False, stop=True, skip_group_check=True)
                    p = p_p.tile([128, 1024], BF, tag="p", name=f"p{ic}{pi}{jb}")
                    nc.scalar.activation(out=_halves(p, off, 512 - off),
                                         in_=_halves(sc, off, 512 - off), func=AF.Exp)
                    if diag and not pe_mask:
                        mk = bass.AP(tensor=misc.tensor, offset=misc.offset + MK0,
                                     ap=[list(misc.ap[0]), [128, 2], [1, 128]])
                        nc.vector.tensor_mul(_halves(p, off, 128),
                                             _halves(p, off, 128), mk)
                    return p

                def pv(ic, pi, jb, po, p):
                    h0, h1 = 2 * pi, 2 * pi + 1
                    off = max(0, 128 * (jb - 4 * ic))
                    vb = vnat[:, jb * 128:(jb + 1) * 128]
                    nc.tensor.matmul(po[h0][:, off:], vb, p[:, off:512],
                                     start=(jb == 0), stop=(jb == 4 * ic + 3))
                    nc.tensor.matmul(po[h1][:, off:], vb, p[:, 512 + off:1024],
                                     start=(jb == 0), stop=(jb == 4 * ic + 3))

                def qk_exp_pv(ic, pi, jb, po):
                    p = qk_exp(ic, pi, jb)
                    pv(ic, pi, jb, po, p)
                    return p

                ost_tiles = {}

                def oproj_chunk(tb, oc, eng):
                    """One [128,512] chunk of the output projection (4 matmuls);
                    per-tb [128,2048] staging, one out DMA per tb."""
                    pso = ps_sc.tile([128, 1024], F32, tag="sc", name=f"os{tb}{oc}")
                    for h in range(H):
                        nc.tensor.matmul(pso[:, 0:512],
                                         oT[h][:, tb * 128:(tb + 1) * 128],
                                         wo[:, h * D + oc * 512:h * D + (oc + 1) * 512],
                                         start=(h == 0), stop=(h == H - 1))
                    if oc == 0:
                        ost_tiles[tb] = ost_p.tile([128, D], BF, tag="ost",
                                                   name=f"ost{tb}")
                    ost = ost_tiles[tb]
                    dst = ost[:, oc * 512:(oc + 1) * 512]
                    if eng == 0:
                        nc.scalar.activation(out=dst, in_=pso[:, 0:512], func=AF.Copy)
                    else:
                        nc.vector.tensor_copy(out=dst, in_=pso[:, 0:512])
                    if tb < 4:
                        # tail row-blocks: store per chunk so the final 1MB of
                        # writes overlaps the last oproj matmuls
                        nc.sync.dma_start(out=out_d[tb * 128:(tb + 1) * 128,
                                                    oc * 512:(oc + 1) * 512], in_=dst)
                    elif oc == 3:
                        nc.sync.dma_start(out=out_d[tb * 128:(tb + 1) * 128, :], in_=ost)

                n_ost = 0
                IC_ORDER = [3, 2, 1, 0]
                for idx, ic in enumerate(IC_ORDER):
                    jb_max = 4 * ic + 3
                    i0 = ic * 512
                    prev_ic = IC_ORDER[idx - 1] if idx > 0 else None
                    pend = ([(tb, oc) for tb in range(4 * prev_ic, 4 * prev_ic + 4)
                             for oc in range(4)] if prev_ic is not None else [])
                    # pair-outer: heads (0,1) sweep all j blocks, then (2,3).
                    # Only 2 PV accumulators live -> 3 rotating score slots.
                    for pi in range(2):
                        po = {h: ps_po.tile([128, 512], F32, tag="acc",
                                            name=f"po{ic}{h}")
                              for h in (2 * pi, 2 * pi + 1)}
                        pts = {}
                        budget = (len(pend) - len(pend) // 2) if pi == 0 else len(pend)
                        # PV lags QK by L blocks so PE never waits on the
                        # exp (ACT) + diagonal-mask (DVE) chain, nor on the
                        # previous pair's po release
                        L = 3
                        pps = []
                        for jb in range(jb_max + 1):
                            pts[jb] = qk_exp(ic, pi, jb)
                            if jb % 2 == 1 and jb < 4 * ic:
                                # pair-sum full tiles on DVE (bf16) so the
                                # denominator ones-chain streams half the columns
                                pp = pp_p.tile([128, 1024], BF, tag="pp",
                                               name=f"pp{ic}{pi}{jb}")
                                nc.vector.tensor_add(pp, pts[jb - 1], pts[jb])
                                pps.append(pp)
                            if jb >= L:
                                pv(ic, pi, jb - L, po, pts[jb - L])
                            if pend and budget > 0 and jb >= (2 if pi == 0 else 1):
                                n = min(-(-budget // (jb_max - jb + 1)), budget)
                                for _ in range(n):
                                    tb, oc = pend.pop(0)
                                    oproj_chunk(tb, oc, n_ost % 2)
                                    n_ost += 1
                                    budget -= 1
                        for jb in range(max(0, jb_max + 1 - L), jb_max + 1):
                            pv(ic, pi, jb, po, pts[jb])
                        # denominator chains + finalize for this pair (overlaps
                        # the other pair's attention on ACT/DVE/GPSIMD)
                        dbs = {}
                        for h in (2 * pi, 2 * pi + 1):
                            half = (h % 2) * 512
                            pdn = ps_sc.tile([1, 512], F32, tag="sc", name=f"pdn{ic}{h}")
                            first = True
                            for pp in pps:
                                nc.tensor.matmul(pdn[:, 0:], ones_bf,
                                                 pp[:, half:half + 512],
                                                 start=first, stop=False)
                                first = False
                            for jb in range(4 * ic, jb_max + 1):
                                off = max(0, 128 * (jb - 4 * ic))
                                nc.tensor.matmul(pdn[:, off:], ones_bf,
                                                 pts[jb][:, half + off:half + 512],
                                                 start=first, stop=(jb == jb_max))
                                first = False
                            drow = dn_p.tile([1, 512], F32, tag="drow",
                                             name=f"drow{ic}{h}")
                            nc.vector.tensor_copy(out=drow, in_=pdn)
                            nc.vector.reciprocal_approx_fast(out=drow, in_=drow)
                            db = db_p.tile([128, 512], F32, tag="db", name=f"db{ic}{h}")
                            nc.gpsimd.partition_broadcast(db, drow, channels=128)
                            dbs[h] = db
                        for h in (2 * pi, 2 * pi + 1):
                            nc.vector.tensor_mul(oT[h][:, i0:i0 + 512], po[h], dbs[h])
                # tail: last processed i-chunk's output projection
                for i, (tb, oc) in enumerate([(tb, oc) for tb in range(0, 4)
                                              for oc in range(4)]):
                    oproj_chunk(tb, oc, i % 2)
    nc.finalize()
    return nc


def _rope_tables():
    d = np.arange(64, dtype=np.float64)
    ang = 10000.0 ** (-d / 64.0)
    pos = np.arange(T, dtype=np.float64)
    rad = pos[None, :] * ang[:, None]          # [64, T]
    cos, sin = np.cos(rad), np.sin(rad)
    cosF = np.concatenate([cos, cos], 0).astype(bf16)
    sinS = np.concatenate([-sin, sin], 0).astype(bf16)
    return np.ascontiguousarray(cosF), np.ascontiguousarray(sinS)


def _pack_kc(w):
    """[D, n] -> [128, KC*n]: block kc at cols [kc*n, (kc+1)*n)."""
    n = w.shape[1]
    out = np.empty((128, KC * n), w.dtype)
    for kc in range(KC):
        out[:, kc * n:(kc + 1) * n] = w[kc * 128:(kc + 1) * 128, :]
    return np.ascontiguousarray(out)


def _in_maps(x, wq, wk, wv, wo, gq, gk):
    cosF, sinS = _rope_tables()
    tri01 = np.triu(np.ones((128, 128), np.float32), 0).astype(bf16)
    mask_pair = np.concatenate([tri01, tri01], 1)          # [128, 256]
    ident = np.eye(128, dtype=bf16)
    maps = []
    for core in range(8):
        b, g = core // 4, core % 4
        wkv_pk = np.concatenate(
            [_pack_kc(wk[:, g * 128:(g + 1) * 128].astype(bf16)),
             _pack_kc(wv[:, g * 128:(g + 1) * 128].astype(bf16))], 1)
        # wq head-major: head h at cols [h*D, (h+1)*D), kc-packed inside
        wq_g = wq[:, g * 512:(g + 1) * 512].astype(bf16)
        wq_pk = np.concatenate(
            [_pack_kc(wq_g[:, h * 128:(h + 1) * 128]) for h in range(H)], 1)
        # wo rows for this group, head h rows -> cols [h*D, (h+1)*D)
        wo_g = wo[g * 512:(g + 1) * 512, :].astype(bf16)
        wo_pk = np.concatenate(
            [wo_g[h * 128:(h + 1) * 128, :] for h in range(H)], 1)
        maskt = np.triu(np.full((128, 128), -1e9, np.float32), 1).astype(bf16)
        misc = np.concatenate([mask_pair, ident, maskt, cosF, sinS], 1)
        assert misc.shape == (128, MISC_W)
        gqk = np.concatenate(
            [(gq[g].T * MULT2).astype(np.float32),
             gk[g].astype(np.float32).reshape(HD, 1)], 1)
        maps.append({
            "xt": np.ascontiguousarray(x[b].T).astype(bf16),
            "wkv": np.ascontiguousarray(wkv_pk),
            "wqp": np.ascontiguousarray(wq_pk),
            "misc": np.ascontiguousarray(misc),
            "gqk": np.ascontiguousarray(gqk),
            "wop": np.ascontiguousarray(wo_pk),
        })
    return maps


def _get_nc():
    if "nc" not in _NC_CACHE:
        _NC_CACHE["nc"] = _build_nc()
    return _NC_CACHE["nc"]


def _run(inputs, trace=False, trace_kwargs=None, tmpdir=None):
    nc = _get_nc()
    maps = _in_maps(inputs["x"], inputs["wq"], inputs["wk"], inputs["wv"],
                    inputs["wo"], inputs["gq"], inputs["gk"])
    res = run_bass_kernel_spmd(nc, maps, core_ids=list(range(8)), trace=trace,
                               tmpdir=tmpdir, **(trace_kwargs or {}))
    out = np.zeros((B, T, D), np.float32)
    for core in range(8):
        out[core // 4] += res.results[core]["out"]
    return out, res


def kernel(**inputs):
    inputs = {k: np.asarray(v) for k, v in inputs.items()}
    out, _ = _run(inputs, trace=False)
    return out


# revision 15
# speedup vs baseline: 1.0560x; 1.0560x over previous
"""Grouped-query attention (B=2,T=2048,D=2048, 4 groups x 4 heads x 128d) on 8 trn2 cores.

Sharding: core = (batch b, group g); b = core//4, g = core%4. Each core computes its
group's QKV projections, QK-rmsnorm+rope, causal flash-style attention, and a partial
output projection o_g @ wo_g; the host sums the 4 per-group partials per batch.

v2 changes over the 306us v1 (all DMA-issue + PE-overhead driven):
  - DMA issue is the phase-1 limiter (~608ns serial issue per dma_start on the Sync
    queue): weights are host-packed into 4KB-row blobs (wkv [128,4096], wq head-major
    [128,8192], misc cos|sin|mask|ident, gqk f32) so all inputs issue in ~27 DMAs
    ordered wkv, xt0..15, wq_h0, misc, gqk, wq_h1..3 -- the xt stream is never
    issue-starved and Q weights land right as K/V proj drains.
  - causal mask moved OFF the PE: post-exp DVE multiply of the diagonal 128-block
    with a 0/1 mask ([128,256] both heads via 3D AP). Kills 64 mask matmuls + 64
    maskt LDWEIGHTS + kb stationary thrash.
  - Q projection kc-outer per head (stationary held 4 matmuls; 64 LDW not 256).
  - output stores staged per-tb [128,2048] -> 16 out DMAs instead of 64 (tail was
    serialized on out-DMA issue).
  - rope swaps stay SBUF->SBUF DMAs on the Sync queue, which is idle after the
    input burst.

Device layout (per core): xt [D,T] bf16 = x[b].T; projections produce qT/kT/vT [n,T]
with head-dim on partitions -> QK^T directly (scores [j,i]); softmax denominator =
ones-matmul chains on PE; PV uses v natural (PE-transposed) as lhsT. All matmuls bf16
with f32 PSUM. Softmax without max-subtraction: |scores| <= sqrt(128) after rmsnorm.
"""

import sys
from contextlib import ExitStack

for _p in ("/opt/trn_rl_repo", "/opt/pypackages"):
    if _p not in sys.path:
        sys.path.insert(0, _p)

import numpy as np
import ml_dtypes

import concourse.bass as bass
import concourse.mybir as mybir
import concourse.tile as tile
from concourse import bacc
from concourse.bass_utils import run_bass_kernel_spmd

bf16 = ml_dtypes.bfloat16
BF = mybir.dt.bfloat16
F32 = mybir.dt.float32
AF = mybir.ActivationFunctionType

B, T, D = 2, 2048, 2048
HD, H, G = 128, 4, 4
KC = D // 128          # 16 contraction chunks
TB = T // 128          # 16 t blocks
IC = T // 512          # 4 i chunks
EPS = 1e-6
MULT2 = float(HD) ** -0.5   # mult^2 folded into q gains

# misc blob column layout: mask_pair | ident | maskt | cos | sin
# (small part [0:512] DMAs first)
MK0, ID0, MT0, COS0, SIN0 = 0, 256, 384, 512, 2560
MISC_W = 4608

_NC_CACHE = {}


def _halves(t, off, width, half=512):
    """3D view of a [128, 2*half] tile: [128, 2, width] starting at `off` in
    each half (covers both heads of a paired tile in one instruction)."""
    return bass.AP(tensor=t.tensor, offset=t.offset + off,
                   ap=[list(t.ap[0]), [half, 2], [1, width]])


def _build_nc():
    nc = bacc.Bacc(None)

    xt_d = nc.declare_dram_parameter("xt", [D, T], BF, isOutput=False)
    wkv_d = nc.declare_dram_parameter("wkv", [128, 2 * D], BF, isOutput=False)
    wq_d = nc.declare_dram_parameter("wqp", [128, H * D], BF, isOutput=False)
    misc_d = nc.declare_dram_parameter("misc", [128, MISC_W], BF, isOutput=False)
    gqk_d = nc.declare_dram_parameter("gqk", [128, 5], F32, isOutput=False)
    wo_d = nc.declare_dram_parameter("wop", [128, H * D], BF, isOutput=False)
    out_d = nc.declare_dram_parameter("out", [T, D], BF, isOutput=True)

    with tile.TileContext(nc) as tc:
        with ExitStack() as outer:
            persist = outer.enter_context(tc.tile_pool(name="persist", bufs=1))
            qhat = [persist.tile([128, T], BF, tag=f"qhat{h}", name=f"qhat{h}") for h in range(H)]
            khat = persist.tile([128, T], BF, tag="khat", name="khat")
            vnat = persist.tile([128, T], BF, tag="vnat", name="vnat")  # [j-local, tb*128+d]
            misc = persist.tile([128, MISC_W], BF, tag="misc", name="misc")
            gqk = persist.tile([128, 5], F32, tag="gqk", name="gqk")
            ones_bf = persist.tile([128, 1], BF, tag="ones", name="ones")
            eps_t = persist.tile([1, 1], F32, tag="eps", name="eps")

            nc.vector.memset(ones_bf, 1.0)
            nc.vector.memset(eps_t, EPS)

            # ---------------- Phase 1: projections + rmsnorm + rope ----------------
            with ExitStack() as s1:
                xt_p = s1.enter_context(tc.tile_pool(name="xt", bufs=1))
                w_p = s1.enter_context(tc.tile_pool(name="w", bufs=1))
                tmp_p = s1.enter_context(tc.tile_pool(name="tmp", bufs=1))
                row_p = s1.enter_context(tc.tile_pool(name="rows", bufs=1))

                xtall = xt_p.tile([128, KC * T], BF, tag="xtall", name="xtall")
                wkv = w_p.tile([128, 2 * D], BF, tag="wkv", name="wkv")
                wqp = w_p.tile([128, H * D], BF, tag="wqp", name="wqp")

                # DMA issue order (one serial issue queue): first K/V weights for
                # kc 0-3 + mask/ident (tiny), then the xt stream (kc=0 split for
                # first-matmul latency), rest of wkv, then Q head 0 weights,
                # cos/sin + gains, then the rest of wq.
                nc.sync.dma_start(out=wkv[:, 0:512], in_=wkv_d[:, 0:512])
                nc.sync.dma_start(out=wkv[:, D:D + 512], in_=wkv_d[:, D:D + 512])
                for q in range(4):
                    nc.sync.dma_start(out=xtall[:, q * 512:(q + 1) * 512],
                                      in_=xt_d[0:128, q * 512:(q + 1) * 512])
                for kc in range(1, 3):
                    nc.sync.dma_start(out=xtall[:, kc * T:(kc + 1) * T],
                                      in_=xt_d[kc * 128:(kc + 1) * 128, :])
                nc.sync.dma_start(out=wkv[:, 512:D], in_=wkv_d[:, 512:D])
                nc.sync.dma_start(out=wkv[:, D + 512:2 * D], in_=wkv_d[:, D + 512:2 * D])
                for kc in range(3, KC):
                    nc.sync.dma_start(out=xtall[:, kc * T:(kc + 1) * T],
                                      in_=xt_d[kc * 128:(kc + 1) * 128, :])
                for q in range(4):
                    nc.sync.dma_start(out=wqp[:, q * 512:(q + 1) * 512],
                                      in_=wq_d[:, q * 512:(q + 1) * 512])
                nc.sync.dma_start(out=misc[:, 0:COS0], in_=misc_d[:, 0:COS0])
                nc.sync.dma_start(out=misc[:, COS0:MISC_W], in_=misc_d[:, COS0:MISC_W])
                nc.sync.dma_start(out=gqk, in_=gqk_d[:, :])
                for h in range(1, H):
                    nc.sync.dma_start(out=wqp[:, h * D:(h + 1) * D],
                                      in_=wq_d[:, h * D:(h + 1) * D])

                def norm_tiles(nm):
                    sq = tmp_p.tile([128, T], BF, tag="sq", name=f"sq_{nm}", bufs=2)
                    gt = tmp_p.tile([128, T], BF, tag="gt", name=f"gt_{nm}", bufs=2)
                    sw = tmp_p.tile([128, T], BF, tag="sw", name=f"sw_{nm}", bufs=2)
                    t1 = tmp_p.tile([128, T], BF, tag="t1", name=f"t1_{nm}", bufs=2)
                    rb = tmp_p.tile([128, T], F32, tag="rb", name=f"rb_{nm}", bufs=2)
                    return sq, gt, sw, t1, rb

                def consume_chunk(ps, sq, gt, sw, t1, gain_col, tf):
                    """square + gain-mul + rotate-half rope, all 512-col chunked."""
                    sl = slice(tf * 512, (tf + 1) * 512)
                    nc.scalar.square(out=sq[:, sl], in_=ps)
                    nc.vector.tensor_scalar_mul(gt[:, sl], ps, gain_col)
                    nc.sync.dma_start(out=sw[0:64, sl], in_=gt[64:128, sl])
                    nc.sync.dma_start(out=sw[64:128, sl], in_=gt[0:64, sl])
                    nc.vector.tensor_mul(t1[:, sl], gt[:, sl],
                                         misc[:, COS0 + tf * 512:COS0 + (tf + 1) * 512])
                    nc.vector.tensor_mul(sw[:, sl], sw[:, sl],
                                         misc[:, SIN0 + tf * 512:SIN0 + (tf + 1) * 512])
                    nc.vector.tensor_add(t1[:, sl], t1[:, sl], sw[:, sl])

                def norm_pe_and_fin(nm, sq, t1, rb, hat_out):
                    """per-chunk: ones-mm -> sqrt -> recip -> gpsimd bcast -> hat.
                    hat lags one chunk so it never waits on the broadcast."""
                    srow = row_p.tile([1, T], F32, tag="srow", name=f"srow_{nm}", bufs=2)
                    prev = None
                    for tf in range(4):
                        sl = slice(tf * 512, (tf + 1) * 512)
                        pr = ps_row.tile([1, 512], F32, tag="ps_row", name=f"pr_{nm}{tf}")
                        nc.tensor.matmul(pr, ones_bf, sq[:, sl], start=True, stop=True)
                        nc.scalar.activation(out=srow[:, sl], in_=pr, func=AF.Sqrt,
                                             bias=eps_t[:, 0:1], scale=1.0 / HD)
                        nc.vector.reciprocal_approx_fast(out=srow[:, sl],
                                                         in_=srow[:, sl])
                        nc.gpsimd.partition_broadcast(rb[:, sl], srow[:, sl],
                                                      channels=128)
                        if prev is not None:
                            nc.vector.tensor_mul(hat_out[:, prev], t1[:, prev],
                                                 rb[:, prev])
                        prev = sl
                    nc.vector.tensor_mul(hat_out[:, prev], t1[:, prev], rb[:, prev])

                # ---- K+V projections, kc-outer interleaved (xt-stream paced) ----
                with ExitStack() as s1a:
                    ps_kv = s1a.enter_context(tc.tile_pool(name="ps_kv", bufs=1, space="PSUM"))
                    psK = [ps_kv.tile([128, 512], F32, tag=f"psK{tf}", name=f"psK{tf}")
                           for tf in range(4)]
                    psV = [ps_kv.tile([128, 512], F32, tag=f"psV{tf}", name=f"psV{tf}")
                           for tf in range(4)]
                    for kc in range(KC):
                        x0 = kc * T
                        for tf in range(4):
                            nc.tensor.matmul(psK[tf], wkv[:, kc * 128:(kc + 1) * 128],
                                             xtall[:, x0 + tf * 512:x0 + (tf + 1) * 512],
                                             start=(kc == 0), stop=(kc == KC - 1))
                        for tf in range(4):
                            nc.tensor.matmul(psV[tf], wkv[:, D + kc * 128:D + (kc + 1) * 128],
                                             xtall[:, x0 + tf * 512:x0 + (tf + 1) * 512],
                                             start=(kc == 0), stop=(kc == KC - 1))
                    sqK, gtK, swK, t1K, rbK = norm_tiles("K")
                    vtr = tmp_p.tile([128, T], BF, tag="vtr", name="vtr")
                    for tf in range(4):
                        consume_chunk(psK[tf], sqK, gtK, swK, t1K, gqk[:, 4:5], tf)
                    for tf in range(4):
                        nc.scalar.activation(out=vtr[:, tf * 512:(tf + 1) * 512],
                                             in_=psV[tf], func=AF.Copy)

                # ---- Q projections kc-outer per head + staggered norms + V transpose ----
                with ExitStack() as s1b:
                    ps_q = s1b.enter_context(tc.tile_pool(name="ps_q", bufs=4, space="PSUM"))
                    ps_row = s1b.enter_context(tc.tile_pool(name="ps_row", bufs=2, space="PSUM"))
                    ps_tp = s1b.enter_context(tc.tile_pool(name="ps_tp", bufs=2, space="PSUM"))

                    qn = [norm_tiles(f"Q{h}") for h in range(H)]

                    def proj_q(h):
                        sq, gt, sw, t1, rb = qn[h]
                        psqs = [ps_q.tile([128, 512], F32, tag="ps_q",
                                          name=f"psq{h}{tf}") for tf in range(4)]
                        for kc in range(KC):
                            w0 = h * D + kc * 128
                            x0 = kc * T
                            for tf in range(4):
                                nc.tensor.matmul(psqs[tf], wqp[:, w0:w0 + 128],
                                                 xtall[:, x0 + tf * 512:x0 + (tf + 1) * 512],
                                                 start=(kc == 0), stop=(kc == KC - 1))
                        for tf in range(4):
                            consume_chunk(psqs[tf], sq, gt, sw, t1, gqk[:, h:h + 1], tf)

                    def vtrans(r):
                        for tb in range(4 * r, 4 * r + 4):
                            pt_ = ps_tp.tile([128, 128], BF, tag="ps_tp", name=f"ps_tp{tb}")
                            nc.tensor.transpose(pt_, vtr[:, tb * 128:(tb + 1) * 128],
                                                misc[:, ID0:ID0 + 128])
                            nc.scalar.activation(out=vnat[:, tb * 128:(tb + 1) * 128],
                                                  in_=pt_, func=AF.Copy)

                    # Q0 projection leads: the K norm chain can only start once
                    # the last xt chunk lands, and putting it first would block
                    # the in-order PE queue on the ACT squares.
                    proj_q(0)
                    norm_pe_and_fin("K", sqK, t1K, rbK, khat)
                    vtrans(0)
                    proj_q(1)
                    norm_pe_and_fin("Q0", qn[0][0], qn[0][3], qn[0][4], qhat[0])
                    vtrans(1)
                    proj_q(2)
                    norm_pe_and_fin("Q1", qn[1][0], qn[1][3], qn[1][4], qhat[1])
                    vtrans(2)
                    proj_q(3)
                    norm_pe_and_fin("Q2", qn[2][0], qn[2][3], qn[2][4], qhat[2])
                    vtrans(3)
                    norm_pe_and_fin("Q3", qn[3][0], qn[3][3], qn[3][4], qhat[3])
                    # prewarm the EXP table so the first attention exp doesn't
                    # pay the 1.3us table switch on the QK->exp critical path
                    warm = row_p.tile([1, 1], F32, tag="warm", name="warm")
                    nc.scalar.activation(out=warm, in_=eps_t, func=AF.Exp)

            # ------- Phases 2+3: causal attention with pipelined output projection ---
            with ExitStack() as s2:
                o_p = s2.enter_context(tc.tile_pool(name="op", bufs=1))
                oT = [o_p.tile([128, T], BF, tag=f"oT{h}", name=f"oT{h}") for h in range(H)]
                wo_p = s2.enter_context(tc.tile_pool(name="wo", bufs=1))
                p_p = s2.enter_context(tc.tile_pool(name="pexp", bufs=26))
                pp_p = s2.enter_context(tc.tile_pool(name="ppair", bufs=8))
                dn_p = s2.enter_context(tc.tile_pool(name="dn", bufs=2))
                db_p = s2.enter_context(tc.tile_pool(name="dnb", bufs=4))
                ost_p = s2.enter_context(tc.tile_pool(name="ost", bufs=3))
                # rotating pool of 3x 2-bank tiles: paired score tiles,
                # denominator rows, and oproj psums all cycle through it.
                # Created before ps_po so its first slots map onto the psum
                # banks freed earliest at the phase-1 handoff.
                ps_sc = s2.enter_context(tc.tile_pool(name="ps_sc", bufs=3, space="PSUM"))
                ps_po = s2.enter_context(tc.tile_pool(name="ps_po", bufs=2, space="PSUM"))

                wo = wo_p.tile([128, H * D], BF, tag="wo", name="wo")
                nc.sync.dma_start(out=wo, in_=wo_d[:, :])

                def qk_exp(ic, pi, jb):
                    """2 QK matmuls into a paired [128,1024] tile, one 3D-AP exp,
                    DVE 0/1-mask multiply on the diagonal block."""
                    h0, h1 = 2 * pi, 2 * pi + 1
                    off = max(0, 128 * (jb - 4 * ic))
                    i0 = ic * 512
                    kb = khat[:, jb * 128:(jb + 1) * 128]
                    diag = jb >= 4 * ic
                    # hybrid causal mask: for the first two i-chunks the DVE
                    # queue is still digesting the Q-norm chains, so the mask is
                    # applied on the PE (accumulated -1e9 matmul); later chunks
                    # use a post-exp DVE 0/1-mask multiply.
                    pe_mask = diag and ic >= 2
                    sc = ps_sc.tile([128, 1024], F32, tag="sc", name=f"sc{ic}{pi}{jb}")
                    nc.tensor.matmul(sc[:, off:512], kb, qhat[h0][:, i0 + off:i0 + 512],
                                     start=True, stop=not pe_mask)
                    nc.tensor.matmul(sc[:, 512 + off:1024], kb,
                                     qhat[h1][:, i0 + off:i0 + 512],
                                     start=True, stop=not pe_mask)
                    if pe_mask:
                        nc.tensor.matmul(sc[:, off:off + 128],
                                         misc[:, MT0:MT0 + 128],
                                         misc[:, ID0:ID0 + 128],
                                         start=False, stop=True, skip_group_check=True)
                        nc.tensor.matmul(sc[:, 512 + off:512 + off + 128],
                                         misc[:, MT0:MT0 + 128],
                                         misc[:, ID0:ID0 + 128],
                                         start=False, stop=True, skip_group_check=True)
                    p = p_p.tile([128, 1024], BF, tag="p", name=f"p{ic}{pi}{jb}")
                    nc.scalar.activation(out=_halves(p, off, 512 - off),
                                         in_=_halves(sc, off, 512 - off), func=AF.Exp)
                    if diag and not pe_mask:
                        mk = bass.AP(tensor=misc.tensor, offset=misc.offset + MK0,
                                     ap=[list(misc.ap[0]), [128, 2], [1, 128]])
                        nc.vector.tensor_mul(_halves(p, off, 128),
                                             _halves(p, off, 128), mk)
                    return p

                def pv(ic, pi, jb, po, p):
                    h0, h1 = 2 * pi, 2 * pi + 1
                    off = max(0, 128 * (jb - 4 * ic))
                    vb = vnat[:, jb * 128:(jb + 1) * 128]
                    nc.tensor.matmul(po[h0][:, off:], vb, p[:, off:512],
                                     start=(jb == 0), stop=(jb == 4 * ic + 3))
                    nc.tensor.matmul(po[h1][:, off:], vb, p[:, 512 + off:1024],
                                     start=(jb == 0), stop=(jb == 4 * ic + 3))

                def qk_exp_pv(ic, pi, jb, po):
                    p = qk_exp(ic, pi, jb)
                    pv(ic, pi, jb, po, p)
                    return p

                ost_tiles = {}

                def oproj_chunk(tb, ocp, eng):
                    """One [128,1024] oc-pair of the output projection (8 matmuls,
                    oT stationary held across the pair); per-tb [128,2048]
                    staging, one out DMA per tb (per-pair for the tail)."""
                    pso = ps_sc.tile([128, 1024], F32, tag="sc", name=f"os{tb}{ocp}")
                    for h in range(H):
                        for q in range(2):
                            oc = 2 * ocp + q
                            nc.tensor.matmul(pso[:, q * 512:(q + 1) * 512],
                                             oT[h][:, tb * 128:(tb + 1) * 128],
                                             wo[:, h * D + oc * 512:h * D + (oc + 1) * 512],
                                             start=(h == 0), stop=(h == H - 1))
                    if ocp == 0:
                        ost_tiles[tb] = ost_p.tile([128, D], BF, tag="ost",
                                                   name=f"ost{tb}")
                    ost = ost_tiles[tb]
                    dst = ost[:, ocp * 1024:(ocp + 1) * 1024]
                    if eng == 0:
                        nc.scalar.activation(out=dst, in_=pso, func=AF.Copy)
                    else:
                        nc.vector.tensor_copy(out=dst, in_=pso)
                    if tb < 4:
                        # tail row-blocks: store per pair so the final 1MB of
                        # writes overlaps the last oproj matmuls
                        nc.sync.dma_start(out=out_d[tb * 128:(tb + 1) * 128,
                                                    ocp * 1024:(ocp + 1) * 1024], in_=dst)
                    elif ocp == 1:
                        nc.sync.dma_start(out=out_d[tb * 128:(tb + 1) * 128, :], in_=ost)

                n_ost = 0
                IC_ORDER = [3, 2, 1, 0]
                for idx, ic in enumerate(IC_ORDER):
                    jb_max = 4 * ic + 3
                    i0 = ic * 512
                    prev_ic = IC_ORDER[idx - 1] if idx > 0 else None
                    pend = ([(tb, ocp) for tb in range(4 * prev_ic, 4 * prev_ic + 4)
                             for ocp in range(2)] if prev_ic is not None else [])
                    # pair-outer: heads (0,1) sweep all j blocks, then (2,3).
                    # Only 2 PV accumulators live -> 3 rotating score slots.
                    for pi in range(2):
                        po = {h: ps_po.tile([128, 512], F32, tag="acc",
                                            name=f"po{ic}{h}")
                              for h in (2 * pi, 2 * pi + 1)}
                        pts = {}
                        budget = (len(pend) - len(pend) // 2) if pi == 0 else len(pend)
                        # PV lags QK by L blocks so PE never waits on the
                        # exp (ACT) + diagonal-mask (DVE) chain, nor on the
                        # previous pair's po release
                        L = 3
                        pps = []
                        for jb in range(jb_max + 1):
                            pts[jb] = qk_exp(ic, pi, jb)
                            if jb % 2 == 1 and jb < 4 * ic:
                                # pair-sum full tiles on DVE (bf16) so the
                                # denominator ones-chain streams half the columns
                                pp = pp_p.tile([128, 1024], BF, tag="pp",
                                               name=f"pp{ic}{pi}{jb}")
                                nc.vector.tensor_add(pp, pts[jb - 1], pts[jb])
                                if jb % 4 == 3:
                                    # second level: 4-tile group sum
                                    pp2 = pp_p.tile([128, 1024], BF, tag="pp",
                                                    name=f"pq{ic}{pi}{jb}")
                                    nc.vector.tensor_add(pp2, pps[-1], pp)
                                    pps[-1] = pp2
                                else:
                                    pps.append(pp)
                            if jb >= L:
                                pv(ic, pi, jb - L, po, pts[jb - L])
                            if pend and budget > 0 and jb >= (2 if pi == 0 else 1):
                                n = min(-(-budget // (jb_max - jb + 1)), budget)
                                for _ in range(n):
                                    tb, ocp = pend.pop(0)
                                    oproj_chunk(tb, ocp, n_ost % 2)
                                    n_ost += 1
                                    budget -= 1
                        for jb in range(max(0, jb_max + 1 - L), jb_max + 1):
                            pv(ic, pi, jb, po, pts[jb])
                        # denominator chains + finalize for this pair (overlaps
                        # the other pair's attention on ACT/DVE/GPSIMD)
                        dbs = {}
                        for h in (2 * pi, 2 * pi + 1):
                            half = (h % 2) * 512
                            pdn = ps_sc.tile([1, 512], F32, tag="sc", name=f"pdn{ic}{h}")
                            first = True
                            for pp in pps:
                                nc.tensor.matmul(pdn[:, 0:], ones_bf,
                                                 pp[:, half:half + 512],
                                                 start=first, stop=False)
                                first = False
                            for jb in range(4 * ic, jb_max + 1):
                                off = max(0, 128 * (jb - 4 * ic))
                                nc.tensor.matmul(pdn[:, off:], ones_bf,
                                                 pts[jb][:, half + off:half + 512],
                                                 start=first, stop=(jb == jb_max))
                                first = False
                            drow = dn_p.tile([1, 512], F32, tag="drow",
                                             name=f"drow{ic}{h}")
                            nc.vector.tensor_copy(out=drow, in_=pdn)
                            nc.vector.reciprocal_approx_fast(out=drow, in_=drow)
                            db = db_p.tile([128, 512], F32, tag="db", name=f"db{ic}{h}")
                            nc.gpsimd.partition_broadcast(db, drow, channels=128)
                            dbs[h] = db
                        for h in (2 * pi, 2 * pi + 1):
                            nc.vector.tensor_mul(oT[h][:, i0:i0 + 512], po[h], dbs[h])
                # tail: last processed i-chunk's output projection
                for i, (tb, ocp) in enumerate([(tb, ocp) for tb in range(0, 4)
                                               for ocp in range(2)]):
                    oproj_chunk(tb, ocp, i % 2)
    nc.finalize()
    return nc


def _rope_tables():
    d = np.arange(64, dtype=np.float64)
    ang = 10000.0 ** (-d / 64.0)
    pos = np.arange(T, dtype=np.float64)
    rad = pos[None, :] * ang[:, None]          # [64, T]
    cos, sin = np.cos(rad), np.sin(rad)
    cosF = np.concatenate([cos, cos], 0).astype(bf16)
    sinS = np.concatenate([-sin, sin], 0).astype(bf16)
    return np.ascontiguousarray(cosF), np.ascontiguousarray(sinS)


def _pack_kc(w):
    """[D, n] -> [128, KC*n]: block kc at cols [kc*n, (kc+1)*n)."""
    n = w.shape[1]
    out = np.empty((128, KC * n), w.dtype)
    for kc in range(KC):
        out[:, kc * n:(kc + 1) * n] = w[kc * 128:(kc + 1) * 128, :]
    return np.ascontiguousarray(out)


def _in_maps(x, wq, wk, wv, wo, gq, gk):
    cosF, sinS = _rope_tables()
    tri01 = np.triu(np.ones((128, 128), np.float32), 0).astype(bf16)
    mask_pair = np.concatenate([tri01, tri01], 1)          # [128, 256]
    ident = np.eye(128, dtype=bf16)
    maps = []
    for core in range(8):
        b, g = core // 4, core % 4
        wkv_pk = np.concatenate(
            [_pack_kc(wk[:, g * 128:(g + 1) * 128].astype(bf16)),
             _pack_kc(wv[:, g * 128:(g + 1) * 128].astype(bf16))], 1)
        # wq head-major: head h at cols [h*D, (h+1)*D), kc-packed inside
        wq_g = wq[:, g * 512:(g + 1) * 512].astype(bf16)
        wq_pk = np.concatenate(
            [_pack_kc(wq_g[:, h * 128:(h + 1) * 128]) for h in range(H)], 1)
        # wo rows for this group, head h rows -> cols [h*D, (h+1)*D)
        wo_g = wo[g * 512:(g + 1) * 512, :].astype(bf16)
        wo_pk = np.concatenate(
            [wo_g[h * 128:(h + 1) * 128, :] for h in range(H)], 1)
        maskt = np.triu(np.full((128, 128), -1e9, np.float32), 1).astype(bf16)
        misc = np.concatenate([mask_pair, ident, maskt, cosF, sinS], 1)
        assert misc.shape == (128, MISC_W)
        gqk = np.concatenate(
            [(gq[g].T * MULT2).astype(np.float32),
             gk[g].astype(np.float32).reshape(HD, 1)], 1)
        maps.append({
            "xt": np.ascontiguousarray(x[b].T).astype(bf16),
            "wkv": np.ascontiguousarray(wkv_pk),
            "wqp": np.ascontiguousarray(wq_pk),
            "misc": np.ascontiguousarray(misc),
            "gqk": np.ascontiguousarray(gqk),
            "wop": np.ascontiguousarray(wo_pk),
        })
    return maps


def _get_nc():
    if "nc" not in _NC_CACHE:
        _NC_CACHE["nc"] = _build_nc()
    return _NC_CACHE["nc"]


def _run(inputs, trace=False, trace_kwargs=None, tmpdir=None):
    nc = _get_nc()
    maps = _in_maps(inputs["x"], inputs["wq"], inputs["wk"], inputs["wv"],
                    inputs["wo"], inputs["gq"], inputs["gk"])
    res = run_bass_kernel_spmd(nc, maps, core_ids=list(range(8)), trace=trace,
                               tmpdir=tmpdir, **(trace_kwargs or {}))
    out = np.zeros((B, T, D), np.float32)
    for core in range(8):
        out[core // 4] += res.results[core]["out"]
    return out, res


def kernel(**inputs):
    inputs = {k: np.asarray(v) for k, v in inputs.items()}
    out, _ = _run(inputs, trace=False)
    return out
